# revision 23
# baseline (speedup 1.0000x reference)
"""BertSelfAttention forward on 8 Trainium2 NeuronCores (Bass/Tile).

Problem: B=2, S=2048, HIDDEN=1024, 16 heads x head_dim 64, fp32 I/O.

Sharding: core c handles batch b = c//4 and head-group g = c%4
(heads 4g..4g+4 == hidden columns 256g..256g+256). Attention is
embarrassingly parallel per (batch, head): no collectives; each core
computes a disjoint [S, 256] slice of the output.

Host-side layout preparation does everything the PE doesn't have to:
  - hs is uploaded PRE-TRANSPOSED and pre-cast: hsT [1024, 2048] bf16.
    This removes every PE transpose and DVE cast from the device program
    and halves the input DMA bytes.
  - W q/k/v are uploaded bf16 in the exact SBUF tile layout [128, 8, 256].
  - bv and the softmax division move to the host: the kernel emits raw
    ctxT = [v | 1].T @ probsT per head ([65, S]: 64 ctx rows + the
    denominator row), written transposed to HBM; the host divides,
    transposes back and adds bv exactly (better than DVE reciprocal).

Per-core device program (matmuls bf16, fp32 PSUM accumulate):
  1. DMA spread over three queues (sync + scalar HWDGE, gpsimd SWDGE),
     W-q/k and the first seq-half of hsT first, so qT/kT dc0 projections
     start ~5us in.
  2. qT/kT [256d, 2048s] = W.T @ hsT (W stationary); bq/bk fused into the
     PSUM->SBUF copies as per-partition DVE scalar-adds. v computed in
     natural [s, d] layout (hsT slab stationary, Wv moving) straight into
     v_sb with a constant-1.0 65th column (softmax denominator trick).
  3. Scores transposed [k, q]: two heads packed into PE rows 0-63/64-127
     (row tiling); per key tile one [128, 1024] psum pair per head-pair.
     exp on ScalarE straight from PSUM with scale=1/8; the additive
     attention mask folds into the per-partition bias (exact reproduction
     of reference masking; all-ones mask -> 0). No max-subtraction:
     scores ~ N(0,1) by construction, exp is safe in fp32 and softmax is
     shift-invariant.
  4. ctxT[65, q] = [v | 1].T @ probsT, v-slice stationary, probs streaming
     at N=512, accumulated over 16 key tiles in PSUM; DVE copies the
     [65, 512] result to SBUF and it DMAs out as-is (host normalizes).

ScalarE's exp stream (~143us) is the target pace; the PE has ~132us of
work, so everything is chopped into ~1-1.3us pieces on a work queue that
the scores/exp stream drains between key tiles. Each emit's ctx runs
during the NEXT emit's stream, except the last emit whose ctx is hooked
into its own stream's tail so only ~2 pieces trail the final exp.
PSUM budget is exactly 16KB/partition: scores 2x4KB + proj/v 2x2KB
(shared tag; v pieces only ever queued between whole proj groups) +
ctx 2x2KB.
"""

import sys
from collections import deque
from contextlib import ExitStack

for _p in ("/opt/trn_rl_repo",):
    if _p not in sys.path:
        sys.path.insert(0, _p)

import ml_dtypes
import numpy as np

import concourse.bass as bass  # noqa: F401
import concourse.mybir as mybir
import concourse.tile as tile
from concourse import bacc
from concourse.bass_utils import run_bass_kernel_spmd

B, S, HID = 2, 2048, 1024
NH, HD = 16, 64
N_CORES = 8
GH = 4  # heads per core
GD = GH * HD  # 256
P = 128
ST = S // P  # 16 seq tiles
HC = HID // P  # 8 hidden chunks
QC = 4  # q chunks of 512
QW = S // QC  # 512
SH = S // 2  # 1024 (hsT half width)
F32 = mybir.dt.float32
BF16 = mybir.dt.bfloat16
EXP = mybir.ActivationFunctionType.Exp
PBUFS = 20  # probs tiles in flight per head-row tag

_CACHE = {}


def _build_nc():
    nc = bacc.Bacc("TRN2", target_bir_lowering=False, debug=False, num_devices=N_CORES)

    hsT_d = nc.dram_tensor("hsT", [HID, S], BF16, kind="ExternalInput").ap()
    wq_d = [
        nc.dram_tensor(f"wq{dc}", [P, HC, P], BF16, kind="ExternalInput").ap()
        for dc in range(2)
    ]
    wk_d = [
        nc.dram_tensor(f"wk{dc}", [P, HC, P], BF16, kind="ExternalInput").ap()
        for dc in range(2)
    ]
    wv_d = nc.dram_tensor("wv", [P, HC, GD], BF16, kind="ExternalInput").ap()
    # packed per-partition smalls: cols 0-1 bq(dc), 2-3 bk(dc), 4-19 mask(kt)
    small_d = nc.dram_tensor("small", [P, 4 + ST], F32, kind="ExternalInput").ap()
    yt_d = nc.dram_tensor("yt", [GH * (HD + 1), S], F32, kind="ExternalOutput").ap()

    with tile.TileContext(nc) as tc:
        with (
            tc.tile_pool(name="const", bufs=1) as constp,
            tc.tile_pool(name="big", bufs=1) as bigp,
            tc.tile_pool(name="probs", bufs=1) as probsp,
            tc.tile_pool(name="outp", bufs=1) as outp,
            tc.tile_pool(name="psS", bufs=1, space="PSUM") as psS,
            tc.tile_pool(name="psC", bufs=1, space="PSUM") as psC,
        ):
            psQ_stack = ExitStack()
            psQ = psQ_stack.enter_context(
                tc.tile_pool(name="psQ", bufs=1, space="PSUM")
            )
            psC2_holder = {}
            wq_sb = [constp.tile([P, HC, P], BF16, name=f"wq{dc}") for dc in range(2)]
            wk_sb = [constp.tile([P, HC, P], BF16, name=f"wk{dc}") for dc in range(2)]
            wv_sb = constp.tile([P, HC, GD], BF16)
            small_sb = constp.tile([P, 4 + ST], F32)
            hsTt = [
                [bigp.tile([P, SH], BF16, name=f"hsT{hc}_{h}") for h in range(2)]
                for hc in range(HC)
            ]
            qTc = [[None] * QC for _ in range(2)]
            kTc = [[None] * QC for _ in range(2)]
            for dc in range(2):
                for sc in range(QC):
                    qTc[dc][sc] = bigp.tile([P, QW], BF16, name=f"qT{dc}_{sc}")
                    kTc[dc][sc] = bigp.tile([P, QW], BF16, name=f"kT{dc}_{sc}")
            v_sb = bigp.tile([P, ST, GH, HD + 1], BF16)

            # ---- DMA issue: W-dc0 + first hsT half prioritized; the
            # second SWDGE ring carries the late-deadline weights ----
            nc.sync.dma_start(wk_sb[0][:], wk_d[0][:])
            nc.scalar.dma_start(wq_sb[0][:], wq_d[0][:])
            nc.gpsimd.dma_start(small_sb[:], small_d[:])

            def hst_dma(eng, hc, h):
                eng.dma_start(
                    hsTt[hc][h][:],
                    hsT_d[hc * P : (hc + 1) * P, h * SH : (h + 1) * SH],
                )

            for hc, h in ((2, 0), (5, 0)):
                hst_dma(nc.gpsimd, hc, h)
            nc.gpsimd.dma_start(wv_sb[:], wv_d[:])
            for hc, h in ((0, 0), (3, 0), (6, 0)):
                hst_dma(nc.sync, hc, h)
            for hc, h in ((1, 0), (4, 0), (7, 0)):
                hst_dma(nc.scalar, hc, h)
            nc.gpsimd.dma_start(wq_sb[1][:], wq_d[1][:])
            nc.gpsimd.dma_start(wk_sb[1][:], wk_d[1][:])
            for hc, h in ((1, 1), (4, 1), (7, 1)):
                hst_dma(nc.sync, hc, h)
            for hc, h in ((2, 1), (5, 1)):
                hst_dma(nc.scalar, hc, h)
            for hc, h in ((0, 1), (3, 1), (6, 1)):
                hst_dma(nc.gpsimd, hc, h)

            # ramp the PE to full p-state before real work arrives (the
            # clock needs ~3us of continuous execution to hit 2.4GHz)
            pewarm = constp.tile([P, QW], BF16)
            nc.vector.memset(pewarm[:], 0.0)
            for i in range(12):
                pool = psQ if i % 2 == 0 else psC
                tag = "ps" if i % 2 == 0 else "ca"
                pw = pool.tile([P, QW], F32, tag=tag, bufs=2, name="pw")
                nc.tensor.matmul(
                    pw[:], lhsT=pewarm[:, 0:P], rhs=pewarm[:], start=True, stop=True
                )
            # pre-load the exp table off the critical path
            warm = constp.tile([P, 1], F32)
            warm2 = constp.tile([P, 1], F32)
            nc.vector.memset(warm[:], 0.0)
            nc.scalar.activation(warm2[:], warm[:], EXP)
            nc.vector.memset(v_sb[:], 1.0)  # col 64 stays 1.0 (denominator)

            # ---- work queue machinery ----
            work = deque()

            def pump(n=None):
                for _ in range(n or 2):
                    if not work:
                        return
                    work.popleft()()

            bank = [0.0]

            def pump_budget(budget=1.9):
                # pop pieces until ~budget us of estimated PE time is queued;
                # keep a small carry so pacing stays smooth across key tiles
                bank[0] = min(bank[0] + budget, 3 * budget)
                while work and getattr(work[0], "_cost", 1.0) <= bank[0]:
                    fn = work.popleft()
                    bank[0] -= getattr(fn, "_cost", 1.0)
                    fn()

            def rhs_for(hc, sc):  # hsT [d-slab, 512 seq] slice
                return hsTt[hc][sc // 2][:, (sc % 2) * QW : (sc % 2 + 1) * QW]

            proj_state = {}

            def proj_quarter(dst_chunks, bias_col, wt, dc, scg, q):
                scs = (2 * scg, 2 * scg + 1)
                key = (bias_col, dc, scg)
                if q == 0:
                    proj_state[key] = [
                        psQ.tile([P, QW], F32, tag="ps", bufs=2, name=f"pp{i}")
                        for i in range(2)
                    ]
                pps = proj_state[key]
                for hc in (2 * q, 2 * q + 1):
                    for i, sc in enumerate(scs):
                        nc.tensor.matmul(
                            pps[i][:],
                            lhsT=wt[:, hc, :],
                            rhs=rhs_for(hc, sc),
                            start=(hc == 0),
                            stop=(hc == HC - 1),
                        )
                if q == 3:
                    for i, sc in enumerate(scs):
                        nc.vector.tensor_scalar_add(
                            out=dst_chunks[sc][:],
                            in0=pps[i][:],
                            scalar1=small_sb[:, bias_col : bias_col + 1],
                        )
                    del proj_state[key]

            def v_piece(st):
                pv = psQ.tile([P, GD], F32, tag="ps", bufs=2, name="pv")
                for hc in range(HC):
                    nc.tensor.matmul(
                        pv[:],
                        lhsT=hsTt[hc][st // 8][:, (st % 8) * P : (st % 8 + 1) * P],
                        rhs=wv_sb[:, hc, :],
                        start=(hc == 0),
                        stop=(hc == HC - 1),
                    )
                nc.vector.tensor_copy(
                    v_sb[:, st, :, 0:HD], pv[:].rearrange("p (h d) -> p h d", d=HD)
                )

            # ---- attention emitters ----
            def scores_emit(pair, qcg, pts, hooks=None):
                q0, q1 = 2 * qcg, 2 * qcg + 1
                for kt in range(ST):
                    if hooks is not None and kt in hooks:
                        hooks[kt]()
                    pump_budget()
                    sc, kk = divmod(kt, 4)
                    for hh, rows, tp in (
                        (0, slice(0, 64), (0, 0)),
                        (1, slice(64, 128), (64, 0)),
                    ):
                        sps = psS.tile([P, 2 * QW], F32, tag=f"s{hh}", bufs=1)
                        for j, qq in ((0, q0), (1, q1)):
                            nc.tensor.matmul(
                                sps[:, j * QW : (j + 1) * QW],
                                lhsT=kTc[pair][sc][rows, kk * P : (kk + 1) * P],
                                rhs=qTc[pair][qq][rows, :],
                                start=True,
                                stop=True,
                                tile_position=tp,
                            )
                        pt = probsp.tile(
                            [P, 2, QW], BF16, tag=f"p{hh}", bufs=PBUFS,
                            name=f"pt{hh}_{kt}",
                        )
                        nc.scalar.activation(
                            pt[:],
                            sps[:].rearrange("p (a b) -> p a b", b=QW),
                            EXP,
                            bias=small_sb[:, 4 + kt : 5 + kt],
                            scale=0.125,
                        )
                        pts[hh].append(pt)

            def ctx_hh_pieces(pair, qcg, pts, hh, use_c2=False):
                """[acc0..acc7 (kt pairs), post_j0, post_j1] for one head row."""
                h = 2 * pair + hh
                pcs = [None, None]
                pieces = []

                def make_accum(kp):
                    def accum():
                        if kp == 0:
                            pool = psC2_holder.get("pool") if use_c2 else psC
                            for j in range(2):
                                pcs[j] = pool.tile(
                                    [HD + 1, QW], F32, tag="ca", bufs=2,
                                    name=f"pc{hh}{j}",
                                )
                        for kt in range(2 * kp, 2 * kp + 2):
                            for j in range(2):
                                nc.tensor.matmul(
                                    pcs[j][:],
                                    lhsT=v_sb[:, kt, h, :],
                                    rhs=pts[hh][kt][:, j],
                                    start=(kt == 0),
                                    stop=(kt == ST - 1),
                                    skip_group_check=True,
                                )

                    return accum

                for kp in range(8):
                    acc = make_accum(kp)
                    acc._cost = 0.9
                    pieces.append(acc)

                def make_post(j):
                    def post():
                        qq = 2 * qcg + j
                        ctxs = outp.tile([HD + 1, QW], F32, tag="ctxs", bufs=4)
                        nc.vector.tensor_copy(ctxs[:], pcs[j][:])
                        nc.sync.dma_start(
                            yt_d[
                                h * (HD + 1) : (h + 1) * (HD + 1),
                                qq * QW : (qq + 1) * QW,
                            ],
                            ctxs[:],
                        )

                    return post

                for j in range(2):
                    p = make_post(j)
                    p._cost = 0.15
                    pieces.append(p)
                return pieces

            # ---- emission ----
            def qpiece(fn):
                fn._is_projv = True
                return fn

            # inline interleaved q/k dc0 scg0: 4 matmuls per hidden chunk,
            # paced to the hsT tile arrivals. k borrows psC's banks (free
            # until e1's ctx, far later).
            ppq = [
                psQ.tile([P, QW], F32, tag="ps", bufs=2, name=f"ppq{i}")
                for i in range(2)
            ]
            ppk = [
                psC.tile([P, QW], F32, tag="ca", bufs=2, name=f"ppk{i}")
                for i in range(2)
            ]
            for hc in range(HC):
                for i, sc in ((0, 0), (1, 1)):
                    nc.tensor.matmul(
                        ppq[i][:], lhsT=wq_sb[0][:, hc, :], rhs=rhs_for(hc, sc),
                        start=(hc == 0), stop=(hc == HC - 1),
                    )
                    nc.tensor.matmul(
                        ppk[i][:], lhsT=wk_sb[0][:, hc, :], rhs=rhs_for(hc, sc),
                        start=(hc == 0), stop=(hc == HC - 1),
                    )
            for i, sc in ((0, 0), (1, 1)):
                nc.vector.tensor_scalar_add(
                    out=qTc[0][sc][:], in0=ppq[i][:], scalar1=small_sb[:, 0:1]
                )
                nc.vector.tensor_scalar_add(
                    out=kTc[0][sc][:], in0=ppk[i][:], scalar1=small_sb[:, 2:3]
                )

            # v st 0-7 only need the first hsT half (early); k0/q0 scg1
            # quarters need the second half, which lands a few us later.
            for st in range(ST // 2):
                work.append(qpiece(lambda st=st: v_piece(st)))
            for q in range(4):
                work.append(qpiece(lambda q=q: proj_quarter(kTc[0], 2, wk_sb[0], 0, 1, q)))
            for q in range(4):
                work.append(qpiece(lambda q=q: proj_quarter(qTc[0], 0, wq_sb[0], 0, 1, q)))
            for st in range(ST // 2, ST):
                work.append(qpiece(lambda st=st: v_piece(st)))

            pts1 = {0: [], 1: []}
            scores_emit(0, 0, pts1)

            for q in range(4):
                work.append(qpiece(lambda q=q: proj_quarter(kTc[1], 3, wk_sb[1], 1, 0, q)))
            for q in range(4):
                work.append(qpiece(lambda q=q: proj_quarter(qTc[1], 1, wq_sb[1], 1, 0, q)))
            work.extend(ctx_hh_pieces(0, 0, pts1, 0))

            pts2 = {0: [], 1: []}
            scores_emit(0, 1, pts2)

            for q in range(4):
                work.append(qpiece(lambda q=q: proj_quarter(kTc[1], 3, wk_sb[1], 1, 1, q)))
            for q in range(4):
                work.append(qpiece(lambda q=q: proj_quarter(qTc[1], 1, wq_sb[1], 1, 1, q)))
            work.extend(ctx_hh_pieces(0, 0, pts1, 1))
            for hh in range(2):
                work.extend(ctx_hh_pieces(0, 1, pts2, hh))

            # stream 3: retire psQ for a second ctx-psum pair early in the
            # stream, then hook e3's first head row into its own stream.
            def pool_switch():  # noqa: ANN202

                while work and (
                    proj_state or any(getattr(f, "_is_projv", False) for f in work)
                ):
                    pump(2)
                psQ_stack.close()
                psC2_holder["pool"] = tc.alloc_tile_pool(
                    name="psC2", bufs=1, space="PSUM"
                )

            pool_switch._cost = 0.0
            pts3 = {0: [], 1: []}
            e3_hh0 = ctx_hh_pieces(1, 0, pts3, 0, use_c2=True)
            hooks3 = {2: pool_switch}
            for i, kt in enumerate((4, 6, 8, 10, 12, 14)):
                hooks3[kt] = lambda i=i: work.append(e3_hh0[i])
            scores_emit(1, 0, pts3, hooks=hooks3)

            work.extend(e3_hh0[6:])  # acc6, acc7, posts
            work.extend(ctx_hh_pieces(1, 0, pts3, 1))

            pts4 = {0: [], 1: []}
            e4_hh0 = ctx_hh_pieces(1, 1, pts4, 0, use_c2=True)
            e4_hh1 = ctx_hh_pieces(1, 1, pts4, 1)
            hooks4 = {}
            for i, kt in enumerate((4, 6, 8, 10, 12, 14)):
                hooks4[kt] = lambda i=i: work.append(e4_hh0[i])
            for i, kt in enumerate((10, 11, 12, 13, 14, 15)):
                prevf = hooks4.get(kt)

                def both(i=i, prevf=prevf):
                    if prevf is not None:
                        prevf()
                    work.append(e4_hh1[i])

                hooks4[kt] = both
            scores_emit(1, 1, pts4, hooks=hooks4)

            while work:
                pump(3)
            for fn in (e4_hh0[6], e4_hh0[7], e4_hh0[8], e4_hh0[9],
                       e4_hh1[6], e4_hh1[7], e4_hh1[8], e4_hh1[9]):
                fn()
            if "pool" in psC2_holder:
                psC2_holder["pool"].release()
    nc.compile()
    return nc


def _make_in_maps(hidden_states, attention_mask, Wq, bq, Wk, bk, Wv, bv):
    min_val = np.finfo(np.float32).min
    hsT_by_b = [
        np.ascontiguousarray(hidden_states[b].T).astype(ml_dtypes.bfloat16)
        for b in range(B)
    ]
    mask_by_b = [
        np.ascontiguousarray(
            ((1.0 - attention_mask[b]) * min_val).astype(np.float32).reshape(ST, P).T
        )
        for b in range(B)
    ]

    def packw(W, sl):
        # [1024, 256] -> [128, 8, 256] so SBUF partition p holds rows p, 128+p, ...
        return np.ascontiguousarray(
            W[:, sl].reshape(HC, P, GD).transpose(1, 0, 2)
        ).astype(ml_dtypes.bfloat16)

    def packw_dc(W, sl, dc):
        return np.ascontiguousarray(packw(W, sl)[:, :, dc * P : (dc + 1) * P])

    in_maps = []
    for c in range(N_CORES):
        b, g = divmod(c, N_CORES // B)
        sl = slice(GD * g, GD * (g + 1))
        small = np.concatenate(
            [bq[sl].reshape(2, P).T, bk[sl].reshape(2, P).T, mask_by_b[b]], axis=1
        ).astype(np.float32)
        in_maps.append(
            {
                "hsT": hsT_by_b[b],
                "wq0": packw_dc(Wq, sl, 0),
                "wq1": packw_dc(Wq, sl, 1),
                "wk0": packw_dc(Wk, sl, 0),
                "wk1": packw_dc(Wk, sl, 1),
                "wv": packw(Wv, sl),
                "small": np.ascontiguousarray(small),
            }
        )
    return in_maps


def kernel(hidden_states, attention_mask, Wq, bq, Wk, bk, Wv, bv):
    hidden_states = np.asarray(hidden_states, dtype=np.float32)
    attention_mask = np.asarray(attention_mask, dtype=np.float32)
    Wq, Wk, Wv = (np.asarray(a, dtype=np.float32) for a in (Wq, Wk, Wv))
    bq, bk, bv = (np.asarray(a, dtype=np.float32) for a in (bq, bk, bv))

    if "nc" not in _CACHE:
        _CACHE["nc"] = _build_nc()
    nc = _CACHE["nc"]

    in_maps = _make_in_maps(hidden_states, attention_mask, Wq, bq, Wk, bk, Wv, bv)
    res = run_bass_kernel_spmd(nc, in_maps, list(range(N_CORES)))
    out = np.empty((B, S, HID), dtype=np.float32)
    for c in range(N_CORES):
        b, g = divmod(c, N_CORES // B)
        ytc = np.asarray(res.results[c]["yt"], dtype=np.float32)  # [260, 2048]
        for h in range(GH):
            blk = ytc[(HD + 1) * h : (HD + 1) * h + HD]  # [64, 2048]
            den = ytc[(HD + 1) * h + HD]  # [2048]
            cols = slice(GD * g + HD * h, GD * g + HD * (h + 1))
            out[b, :, cols] = (blk / den).T + bv[cols]
    return out


# revision 28
# speedup vs baseline: 1.1901x; 1.1901x over previous
"""BertSelfAttention forward on 8 Trainium2 NeuronCores (Bass/Tile).

Problem: B=2, S=2048, HIDDEN=1024, 16 heads x head_dim 64, fp32 I/O.

Sharding: core c handles batch b = c//4 and head-group g = c%4
(heads 4g..4g+4 == hidden columns 256g..256g+256). Attention is
embarrassingly parallel per (batch, head): no collectives; each core
computes a disjoint [S, 256] slice of the output.

Host-side layout preparation does everything the PE doesn't have to:
  - hs is uploaded PRE-TRANSPOSED and pre-cast: hsT [1024, 2048] bf16.
    This removes every PE transpose and DVE cast from the device program
    and halves the input DMA bytes.
  - W q/k/v are uploaded bf16 in the exact SBUF tile layout [128, 8, 256].
  - bv and the softmax division move to the host: the kernel emits raw
    ctxT = [v | 1].T @ probsT per head ([65, S]: 64 ctx rows + the
    denominator row), written transposed to HBM; the host divides,
    transposes back and adds bv exactly (better than DVE reciprocal).

Per-core device program (matmuls bf16, fp32 PSUM accumulate):
  1. DMA spread over three queues (sync + scalar HWDGE, gpsimd SWDGE),
     W-q/k and the first seq-half of hsT first, so qT/kT dc0 projections
     start ~5us in.
  2. qT/kT [256d, 2048s] = W.T @ hsT (W stationary); bq/bk fused into the
     PSUM->SBUF copies as per-partition DVE scalar-adds. v computed in
     natural [s, d] layout (hsT slab stationary, Wv moving) straight into
     v_sb with a constant-1.0 65th column (softmax denominator trick).
  3. Scores transposed [k, q]: two heads packed into PE rows 0-63/64-127
     (row tiling); per key tile one [128, 1024] psum pair per head-pair.
     exp on ScalarE straight from PSUM with scale=1/8; the additive
     attention mask folds into the per-partition bias (exact reproduction
     of reference masking; all-ones mask -> 0). No max-subtraction:
     scores ~ N(0,1) by construction, exp is safe in fp32 and softmax is
     shift-invariant.
  4. ctxT[65, q] = [v | 1].T @ probsT, v-slice stationary, probs streaming
     at N=512, accumulated over 16 key tiles in PSUM; DVE copies the
     [65, 512] result to SBUF and it DMAs out as-is (host normalizes).

ScalarE's exp stream (~143us) is the target pace; the PE has ~132us of
work, so everything is chopped into ~1-1.3us pieces on a work queue that
the scores/exp stream drains between key tiles. Each emit's ctx runs
during the NEXT emit's stream, except the last emit whose ctx is hooked
into its own stream's tail so only ~2 pieces trail the final exp.
PSUM budget is exactly 16KB/partition: scores 2x4KB + proj/v 2x2KB
(shared tag; v pieces only ever queued between whole proj groups) +
ctx 2x2KB.
"""

import sys
from collections import deque
from contextlib import ExitStack

for _p in ("/opt/trn_rl_repo",):
    if _p not in sys.path:
        sys.path.insert(0, _p)

import ml_dtypes
import numpy as np

import concourse.bass as bass  # noqa: F401
import concourse.mybir as mybir
import concourse.tile as tile
from concourse import bacc
from concourse.bass_utils import run_bass_kernel_spmd

B, S, HID = 2, 2048, 1024
NH, HD = 16, 64
N_CORES = 8
GH = 4  # heads per core
GD = GH * HD  # 256
P = 128
ST = S // P  # 16 seq tiles
HC = HID // P  # 8 hidden chunks
QC = 4  # q chunks of 512
QW = S // QC  # 512
SH = S // 2  # 1024 (hsT half width)
F32 = mybir.dt.float32
BF16 = mybir.dt.bfloat16
EXP = mybir.ActivationFunctionType.Exp
PBUFS = 20  # probs tiles in flight per head-row tag

_CACHE = {}


def _build_nc():
    nc = bacc.Bacc("TRN2", target_bir_lowering=False, debug=False, num_devices=N_CORES)

    hsT_d = nc.dram_tensor("hsT", [HID, S], BF16, kind="ExternalInput").ap()
    wq_d = [
        nc.dram_tensor(f"wq{dc}", [P, HC, P], BF16, kind="ExternalInput").ap()
        for dc in range(2)
    ]
    wk_d = [
        nc.dram_tensor(f"wk{dc}", [P, HC, P], BF16, kind="ExternalInput").ap()
        for dc in range(2)
    ]
    wv_d = nc.dram_tensor("wv", [P, HC, GD], BF16, kind="ExternalInput").ap()
    # packed per-partition smalls: cols 0-1 bq(dc), 2-3 bk(dc), 4-19 mask(kt)
    small_d = nc.dram_tensor("small", [P, 4 + ST], F32, kind="ExternalInput").ap()
    yt_d = nc.dram_tensor("yt", [GH * (HD + 1), S], F32, kind="ExternalOutput").ap()

    with tile.TileContext(nc) as tc:
        with (
            tc.tile_pool(name="const", bufs=1) as constp,
            tc.tile_pool(name="big", bufs=1) as bigp,
            tc.tile_pool(name="probs", bufs=1) as probsp,
            tc.tile_pool(name="outp", bufs=1) as outp,
            tc.tile_pool(name="psS", bufs=1, space="PSUM") as psS,
            tc.tile_pool(name="psC", bufs=1, space="PSUM") as psC,
        ):
            psQ_stack = ExitStack()
            psQ = psQ_stack.enter_context(
                tc.tile_pool(name="psQ", bufs=1, space="PSUM")
            )
            psC2_holder = {}
            wq_sb = [constp.tile([P, HC, P], BF16, name=f"wq{dc}") for dc in range(2)]
            wk_sb = [constp.tile([P, HC, P], BF16, name=f"wk{dc}") for dc in range(2)]
            wv_sb = constp.tile([P, HC, GD], BF16)
            small_sb = constp.tile([P, 4 + ST], F32)
            hsTt = [
                [bigp.tile([P, SH], BF16, name=f"hsT{hc}_{h}") for h in range(2)]
                for hc in range(HC)
            ]
            qTc = [[None] * QC for _ in range(2)]
            kTc = [[None] * QC for _ in range(2)]
            for dc in range(2):
                for sc in range(QC):
                    qTc[dc][sc] = bigp.tile([P, QW], BF16, name=f"qT{dc}_{sc}")
                    kTc[dc][sc] = bigp.tile([P, QW], BF16, name=f"kT{dc}_{sc}")
            v_sb = bigp.tile([P, ST, GH, HD + 1], BF16)

            # ---- DMA issue: W-dc0 + first hsT half prioritized; the
            # second SWDGE ring carries the late-deadline weights ----
            nc.sync.dma_start(wk_sb[0][:], wk_d[0][:])
            nc.scalar.dma_start(wq_sb[0][:], wq_d[0][:])
            nc.gpsimd.dma_start(small_sb[:], small_d[:])

            def hst_dma(eng, hc, h):
                eng.dma_start(
                    hsTt[hc][h][:],
                    hsT_d[hc * P : (hc + 1) * P, h * SH : (h + 1) * SH],
                )

            for hc, h in ((2, 0), (5, 0)):
                hst_dma(nc.gpsimd, hc, h)
            nc.gpsimd.dma_start(wv_sb[:], wv_d[:])
            for hc, h in ((0, 0), (3, 0), (6, 0)):
                hst_dma(nc.sync, hc, h)
            for hc, h in ((1, 0), (4, 0), (7, 0)):
                hst_dma(nc.scalar, hc, h)
            nc.gpsimd.dma_start(wq_sb[1][:], wq_d[1][:])
            nc.gpsimd.dma_start(wk_sb[1][:], wk_d[1][:])
            for hc, h in ((1, 1), (4, 1), (7, 1)):
                hst_dma(nc.sync, hc, h)
            for hc, h in ((2, 1), (5, 1)):
                hst_dma(nc.scalar, hc, h)
            for hc, h in ((0, 1), (3, 1), (6, 1)):
                hst_dma(nc.gpsimd, hc, h)

            # ramp the PE to full p-state before real work arrives (the
            # clock needs ~3us of continuous execution to hit 2.4GHz)
            pewarm = constp.tile([P, QW], BF16)
            nc.vector.memset(pewarm[:], 0.0)
            for i in range(12):
                pool = psQ if i % 2 == 0 else psC
                tag = "ps" if i % 2 == 0 else "ca"
                pw = pool.tile([P, QW], F32, tag=tag, bufs=2, name="pw")
                nc.tensor.matmul(
                    pw[:], lhsT=pewarm[:, 0:P], rhs=pewarm[:], start=True, stop=True
                )
            # pre-load the exp table off the critical path
            warm = constp.tile([P, 1], F32)
            warm2 = constp.tile([P, 1], F32)
            nc.vector.memset(warm[:], 0.0)
            nc.scalar.activation(warm2[:], warm[:], EXP)
            nc.vector.memset(v_sb[:], 1.0)  # col 64 stays 1.0 (denominator)

            # ---- work queue machinery ----
            work = deque()

            def pump(n=None):
                for _ in range(n or 2):
                    if not work:
                        return
                    work.popleft()()

            bank = [0.0]

            def pump_budget(budget=1.9):
                # pop pieces until ~budget us of estimated PE time is queued;
                # keep a small carry so pacing stays smooth across key tiles
                bank[0] = min(bank[0] + budget, 3 * budget)
                while work and getattr(work[0], "_cost", 1.0) <= bank[0]:
                    fn = work.popleft()
                    bank[0] -= getattr(fn, "_cost", 1.0)
                    fn()

            def rhs_for(hc, sc):  # hsT [d-slab, 512 seq] slice
                return hsTt[hc][sc // 2][:, (sc % 2) * QW : (sc % 2 + 1) * QW]

            proj_state = {}

            def proj_eighth(dst_chunks, bias_col, wt, dc, scg, hc):
                scs = (2 * scg, 2 * scg + 1)
                key = (bias_col, dc, scg)
                if hc == 0:
                    proj_state[key] = [
                        psQ.tile([P, QW], F32, tag="ps", bufs=2, name=f"pp{i}")
                        for i in range(2)
                    ]
                pps = proj_state[key]
                for i, sc in enumerate(scs):
                    nc.tensor.matmul(
                        pps[i][:],
                        lhsT=wt[:, hc, :],
                        rhs=rhs_for(hc, sc),
                        start=(hc == 0),
                        stop=(hc == HC - 1),
                    )
                if hc == HC - 1:
                    for i, sc in enumerate(scs):
                        nc.vector.tensor_scalar_add(
                            out=dst_chunks[sc][:],
                            in0=pps[i][:],
                            scalar1=small_sb[:, bias_col : bias_col + 1],
                        )
                    del proj_state[key]

            v_state = {}

            def v_half(st, half):
                if half == 0:
                    v_state[st] = psQ.tile([P, GD], F32, tag="ps", bufs=2, name="pv")
                pv = v_state[st]
                for hc in range(4 * half, 4 * half + 4):
                    nc.tensor.matmul(
                        pv[:],
                        lhsT=hsTt[hc][st // 8][:, (st % 8) * P : (st % 8 + 1) * P],
                        rhs=wv_sb[:, hc, :],
                        start=(hc == 0),
                        stop=(hc == HC - 1),
                    )
                if half == 1:
                    nc.vector.tensor_copy(
                        v_sb[:, st, :, 0:HD],
                        pv[:].rearrange("p (h d) -> p h d", d=HD),
                    )
                    del v_state[st]

            # ---- attention emitters ----
            def scores_emit(pair, qcg, pts, hooks=None):
                q0, q1 = 2 * qcg, 2 * qcg + 1
                for kt in range(ST):
                    if hooks is not None and kt in hooks:
                        hooks[kt]()
                    pump_budget()
                    sc, kk = divmod(kt, 4)
                    for hh, rows, tp in (
                        (0, slice(0, 64), (0, 0)),
                        (1, slice(64, 128), (64, 0)),
                    ):
                        sps = psS.tile([P, 2 * QW], F32, tag=f"s{hh}", bufs=1)
                        for j, qq in ((0, q0), (1, q1)):
                            nc.tensor.matmul(
                                sps[:, j * QW : (j + 1) * QW],
                                lhsT=kTc[pair][sc][rows, kk * P : (kk + 1) * P],
                                rhs=qTc[pair][qq][rows, :],
                                start=True,
                                stop=True,
                                tile_position=tp,
                            )
                        pt = probsp.tile(
                            [P, 2, QW], BF16, tag=f"p{hh}", bufs=PBUFS,
                            name=f"pt{hh}_{kt}",
                        )
                        nc.scalar.activation(
                            pt[:],
                            sps[:].rearrange("p (a b) -> p a b", b=QW),
                            EXP,
                            bias=small_sb[:, 4 + kt : 5 + kt],
                            scale=0.125,
                        )
                        pts[hh].append(pt)

            def ctx_hh_pieces(pair, qcg, pts, hh, use_c2=False):
                """[acc0..acc7 (kt pairs), post_j0, post_j1] for one head row."""
                h = 2 * pair + hh
                pcs = [None, None]
                pieces = []

                def make_accum(kp):
                    def accum():
                        if kp == 0:
                            pool = psC2_holder.get("pool") if use_c2 else psC
                            for j in range(2):
                                pcs[j] = pool.tile(
                                    [HD + 1, QW], F32, tag="ca", bufs=2,
                                    name=f"pc{hh}{j}",
                                )
                        for kt in range(2 * kp, 2 * kp + 2):
                            for j in range(2):
                                nc.tensor.matmul(
                                    pcs[j][:],
                                    lhsT=v_sb[:, kt, h, :],
                                    rhs=pts[hh][kt][:, j],
                                    start=(kt == 0),
                                    stop=(kt == ST - 1),
                                    skip_group_check=True,
                                )

                    return accum

                for kp in range(8):
                    acc = make_accum(kp)
                    acc._cost = 0.9
                    pieces.append(acc)

                def make_post(j):
                    def post():
                        qq = 2 * qcg + j
                        ctxs = outp.tile([HD + 1, QW], F32, tag="ctxs", bufs=4)
                        nc.vector.tensor_copy(ctxs[:], pcs[j][:])
                        nc.sync.dma_start(
                            yt_d[
                                h * (HD + 1) : (h + 1) * (HD + 1),
                                qq * QW : (qq + 1) * QW,
                            ],
                            ctxs[:],
                        )

                    return post

                for j in range(2):
                    p = make_post(j)
                    p._cost = 0.15
                    pieces.append(p)
                return pieces

            # ---- emission ----
            def qpiece(fn):
                fn._is_projv = True
                return fn

            # inline interleaved q/k dc0 scg0: 4 matmuls per hidden chunk,
            # paced to the hsT tile arrivals. k borrows psC's banks (free
            # until e1's ctx, far later).
            ppq = [
                psQ.tile([P, QW], F32, tag="ps", bufs=2, name=f"ppq{i}")
                for i in range(2)
            ]
            ppk = [
                psC.tile([P, QW], F32, tag="ca", bufs=2, name=f"ppk{i}")
                for i in range(2)
            ]
            for hc in range(HC):
                for i, sc in ((0, 0), (1, 1)):
                    nc.tensor.matmul(
                        ppq[i][:], lhsT=wq_sb[0][:, hc, :], rhs=rhs_for(hc, sc),
                        start=(hc == 0), stop=(hc == HC - 1),
                    )
                    nc.tensor.matmul(
                        ppk[i][:], lhsT=wk_sb[0][:, hc, :], rhs=rhs_for(hc, sc),
                        start=(hc == 0), stop=(hc == HC - 1),
                    )
            for i, sc in ((0, 0), (1, 1)):
                nc.vector.tensor_scalar_add(
                    out=qTc[0][sc][:], in0=ppq[i][:], scalar1=small_sb[:, 0:1]
                )
                nc.vector.tensor_scalar_add(
                    out=kTc[0][sc][:], in0=ppk[i][:], scalar1=small_sb[:, 2:3]
                )

            def add_v(st):
                for half in range(2):
                    f = qpiece(lambda st=st, half=half: v_half(st, half))
                    f._cost = 0.55
                    work.append(f)

            def add_proj(dst, bias_col, wt, dc, scg):
                for hc in range(HC):
                    f = qpiece(
                        lambda hc=hc: proj_eighth(dst, bias_col, wt, dc, scg, hc)
                    )
                    f._cost = 0.55
                    work.append(f)

            # v st 0-7 only need the first hsT half (early); k0/q0 scg1
            # pieces need the second half, which lands a few us later.
            for st in range(ST // 2):
                add_v(st)
            add_proj(kTc[0], 2, wk_sb[0], 0, 1)
            add_proj(qTc[0], 0, wq_sb[0], 0, 1)
            for st in range(ST // 2, ST):
                add_v(st)

            pts1 = {0: [], 1: []}
            scores_emit(0, 0, pts1)

            add_proj(kTc[1], 3, wk_sb[1], 1, 0)
            add_proj(qTc[1], 1, wq_sb[1], 1, 0)
            add_proj(kTc[1], 3, wk_sb[1], 1, 1)
            add_proj(qTc[1], 1, wq_sb[1], 1, 1)
            work.extend(ctx_hh_pieces(0, 0, pts1, 0))

            pts2 = {0: [], 1: []}
            scores_emit(0, 1, pts2)

            work.extend(ctx_hh_pieces(0, 0, pts1, 1))
            for hh in range(2):
                work.extend(ctx_hh_pieces(0, 1, pts2, hh))

            # stream 3: retire psQ for a second ctx-psum pair early in the
            # stream, then hook e3's first head row into its own stream.
            def pool_switch():  # noqa: ANN202

                while work and (
                    proj_state or any(getattr(f, "_is_projv", False) for f in work)
                ):
                    pump(2)
                psQ_stack.close()
                psC2_holder["pool"] = tc.alloc_tile_pool(
                    name="psC2", bufs=1, space="PSUM"
                )

            pool_switch._cost = 0.0
            pts3 = {0: [], 1: []}
            e3_hh0 = ctx_hh_pieces(1, 0, pts3, 0, use_c2=True)
            hooks3 = {2: pool_switch}
            for i, kt in enumerate((4, 6, 8, 10, 12, 14)):
                hooks3[kt] = lambda i=i: work.append(e3_hh0[i])
            scores_emit(1, 0, pts3, hooks=hooks3)

            work.extend(e3_hh0[6:])  # acc6, acc7, posts
            work.extend(ctx_hh_pieces(1, 0, pts3, 1))

            pts4 = {0: [], 1: []}
            e4_hh0 = ctx_hh_pieces(1, 1, pts4, 0, use_c2=True)
            e4_hh1 = ctx_hh_pieces(1, 1, pts4, 1)
            hooks4 = {}
            for i, kt in enumerate((4, 6, 8, 10, 12, 14)):
                hooks4[kt] = lambda i=i: work.append(e4_hh0[i])
            for i, kt in enumerate((10, 11, 12, 13, 14, 15)):
                prevf = hooks4.get(kt)

                def both(i=i, prevf=prevf):
                    if prevf is not None:
                        prevf()
                    work.append(e4_hh1[i])

                hooks4[kt] = both
            scores_emit(1, 1, pts4, hooks=hooks4)

            while work:
                pump(3)
            for fn in (e4_hh0[6], e4_hh0[7], e4_hh0[8], e4_hh0[9],
                       e4_hh1[6], e4_hh1[7], e4_hh1[8], e4_hh1[9]):
                fn()
            if "pool" in psC2_holder:
                psC2_holder["pool"].release()
    nc.compile()
    return nc


def _make_in_maps(hidden_states, attention_mask, Wq, bq, Wk, bk, Wv, bv):
    min_val = np.finfo(np.float32).min
    hsT_by_b = [
        np.ascontiguousarray(hidden_states[b].T).astype(ml_dtypes.bfloat16)
        for b in range(B)
    ]
    mask_by_b = [
        np.ascontiguousarray(
            ((1.0 - attention_mask[b]) * min_val).astype(np.float32).reshape(ST, P).T
        )
        for b in range(B)
    ]

    def packw(W, sl):
        # [1024, 256] -> [128, 8, 256] so SBUF partition p holds rows p, 128+p, ...
        return np.ascontiguousarray(
            W[:, sl].reshape(HC, P, GD).transpose(1, 0, 2)
        ).astype(ml_dtypes.bfloat16)

    def packw_dc(W, sl, dc):
        return np.ascontiguousarray(packw(W, sl)[:, :, dc * P : (dc + 1) * P])

    in_maps = []
    for c in range(N_CORES):
        b, g = divmod(c, N_CORES // B)
        sl = slice(GD * g, GD * (g + 1))
        small = np.concatenate(
            [bq[sl].reshape(2, P).T, bk[sl].reshape(2, P).T, mask_by_b[b]], axis=1
        ).astype(np.float32)
        in_maps.append(
            {
                "hsT": hsT_by_b[b],
                "wq0": packw_dc(Wq, sl, 0),
                "wq1": packw_dc(Wq, sl, 1),
                "wk0": packw_dc(Wk, sl, 0),
                "wk1": packw_dc(Wk, sl, 1),
                "wv": packw(Wv, sl),
                "small": np.ascontiguousarray(small),
            }
        )
    return in_maps


def kernel(hidden_states, attention_mask, Wq, bq, Wk, bk, Wv, bv):
    hidden_states = np.asarray(hidden_states, dtype=np.float32)
    attention_mask = np.asarray(attention_mask, dtype=np.float32)
    Wq, Wk, Wv = (np.asarray(a, dtype=np.float32) for a in (Wq, Wk, Wv))
    bq, bk, bv = (np.asarray(a, dtype=np.float32) for a in (bq, bk, bv))

    if "nc" not in _CACHE:
        _CACHE["nc"] = _build_nc()
    nc = _CACHE["nc"]

    in_maps = _make_in_maps(hidden_states, attention_mask, Wq, bq, Wk, bk, Wv, bv)
    res = run_bass_kernel_spmd(nc, in_maps, list(range(N_CORES)))
    out = np.empty((B, S, HID), dtype=np.float32)
    for c in range(N_CORES):
        b, g = divmod(c, N_CORES // B)
        ytc = np.asarray(res.results[c]["yt"], dtype=np.float32)  # [260, 2048]
        for h in range(GH):
            blk = ytc[(HD + 1) * h : (HD + 1) * h + HD]  # [64, 2048]
            den = ytc[(HD + 1) * h + HD]  # [2048]
            cols = slice(GD * g + HD * h, GD * g + HD * (h + 1))
            out[b, :, cols] = (blk / den).T + bv[cols]
    return out
